# revision 2
# baseline (speedup 1.0000x reference)
"""Trainium2 Bass kernel for nn_BasicAttention (B=8, C=1024, L=2048, A=128).

Sharding: data-parallel over batch B — one example per NeuronCore, no
collectives.

Math (per example), associativity avoids any on-device transpose:
    keys    = Wk @ x + bk                      [A, L]
    queries = Wq @ x + bq                      [A, L]
    V       = keys^T @ queries                 [L, L]
    E       = exp(V / (L/2))   (raw exp; logits are ~1e-2 so no max-sub)
    S[l]    = sum_m E[l, m]
    yT      = x^T @ Wp^T       (= (Wp @ x)^T)  [L, C]
    out     = (yT / S)^T @ E + bp              [C, L]

v4 = the original static-PE-address discipline (every matmul operand AP
is compile-time static; register offsets appear only on DMA and
activation APs, which tolerate them) + two structural wins:
  * all GEMMs in bf16 (same PE rate as f32r here, half the bytes), so
  * E = exp(V) lives entirely in SBUF (64 KiB/partition) — phase 2 no
    longer round-trips 32 MB/core through DRAM, and phase 3 stages E
    chunks via cheap SBUF->SBUF DMA instead of DRAM reads.
Phase 2 processes 4 l-tiles per loop body so the x l-block staging DMA
is one contiguous copy amortized over 4x the matmul work, split in two
halves so the next body's staging overlaps this body's tail matmuls.

Layouts (per partition p):
    x_sb  col = mc*4096 + c*512 + j       (mc m-chunk, c c-tile, j in chunk)
    wp_sb col = c*1024 + d_out            (wpT, c-tile-major)
    kq_sb col = c*128 + a (wkT) then +1024 same for wqT
    E_sb  tile l=4*mc+u in slot t=(u*4+mc), col t*2048
    yt_sb tile l=4*mc+u in slot t=(u*4+mc), col t*1024
"""

import os
import sys

for _p in ("/opt/trn_rl_repo", "/root/.axon_site/_ro/trn_rl_repo"):
    if os.path.isdir(_p) and _p not in sys.path:
        sys.path.insert(0, _p)

import numpy as np
import ml_dtypes
from contextlib import ExitStack

from concourse import bass, bacc, mybir, tile
from concourse.bass_utils import run_bass_kernel_spmd

P = 128
B, C, L, A = 8, 1024, 2048, 128
NC_TILES = C // P          # 8 c-tiles
NL_TILES = L // P          # 16 l-tiles
ND_TILES = C // P          # 8 d-tiles
NCHUNK = 512
NMCH = L // NCHUNK         # 4 m-chunks
XCH = NC_TILES * NCHUNK    # x cols per m-chunk (4096)

F32 = mybir.dt.float32
BF16 = mybir.dt.bfloat16
AF = mybir.ActivationFunctionType
ds = bass.ds

_NC_CACHE = {}


def build_nc(rep: int = 1):
    SR = os.environ.get('KERNEL_SR', '1') == '1'
    PH = os.environ.get('BENCH_PHASES', '123')
    REP_SR = os.environ.get('REP_SR', '0') == '1'
    nc = bacc.Bacc(None, target_bir_lowering=False)

    x_d = nc.declare_dram_parameter("x", [P, NMCH * XCH], BF16, isOutput=False)
    wp_d = nc.declare_dram_parameter("wp", [P, NC_TILES * C], BF16, isOutput=False)
    kq_d = nc.declare_dram_parameter("kq", [P, 2 * NC_TILES * A], BF16, isOutput=False)
    b_d = nc.declare_dram_parameter("b", [P, 2 + ND_TILES], F32, isOutput=False)
    out_d = nc.declare_dram_parameter("out", [C, L], F32, isOutput=True)

    with tile.TileContext(nc) as tc, ExitStack() as octx:
        big = octx.enter_context(tc.tile_pool(name="big", bufs=1))
        x_sb = big.tile([P, NMCH * XCH], BF16)
        wp_sb = big.tile([P, NC_TILES * C], BF16)
        kq_sb = big.tile([P, 2 * NC_TILES * A], BF16)
        b_sb = big.tile([P, 2 + ND_TILES], F32)
        keys_sb = big.tile([P, L], BF16)
        quer_sb = big.tile([P, L], BF16)
        E_sb = big.tile([P, NL_TILES * L], BF16)
        yt_sb = big.tile([P, NL_TILES * C], BF16)
        k_stage = big.tile([P, NCHUNK], BF16)
        xl_a = big.tile([P, NC_TILES, 2 * P], BF16)   # u=0,1 halves
        xl_b = big.tile([P, NC_TILES, 2 * P], BF16)   # u=2,3 halves
        a_stage = big.tile([P, NL_TILES, NCHUNK], BF16)
        s_st = big.tile([P, NMCH], F32)
        rs_st = big.tile([P, NMCH], F32)

        nc.sync.dma_start(out=kq_sb[:], in_=kq_d[:])
        nc.sync.dma_start(out=b_sb[:], in_=b_d[:])
        for mc in range(NMCH):
            nc.sync.dma_start(out=x_sb[:, mc * XCH:(mc + 1) * XCH],
                              in_=x_d[:, mc * XCH:(mc + 1) * XCH])
        nc.sync.dma_start(out=wp_sb[:], in_=wp_d[:])

        # x viewed as [p, mc, c, u-half, 256]
        x4 = x_sb.rearrange("p (m c h q) -> p m c h q",
                            m=NMCH, c=NC_TILES, h=2)

        rep_ctx = (tc.For_i(0, rep, 1, staggered_reset=REP_SR)
                   if rep > 1 else None)
        if rep_ctx is not None:
            rep_ctx.__enter__()

        # ============ L1: K/Q projections (4 iters) ============
        ps1 = tc.alloc_tile_pool(name="ps1", bufs=2, space="PSUM")
        if "1" in PH:
          with tc.For_i(0, NMCH, 1, staggered_reset=SR) as mc:
            for woff, bcol, o_sb in ((0, 0, keys_sb), (C, 1, quer_sb)):
                acc = ps1.tile([P, NCHUNK], F32, tag="ps1",
                               name="accK" if woff == 0 else "accQ")
                for c in range(NC_TILES):
                    nc.tensor.matmul(
                        out=acc[:],
                        lhsT=kq_sb[:, woff + c * A:woff + (c + 1) * A],
                        rhs=x_sb[:, ds(mc * XCH + c * NCHUNK, NCHUNK)],
                        start=(c == 0), stop=(c == NC_TILES - 1))
                nc.scalar.activation(
                    o_sb[:, ds(mc * NCHUNK, NCHUNK)], acc[:],
                    AF.Identity, bias=b_sb[:, bcol:bcol + 1])
        ps1.release()

        # ==== L2: values + exp + yT (4 iters, 4 l-tiles each) ====
        ps23 = tc.alloc_tile_pool(name="ps23", bufs=2, space="PSUM")
        if "2" in PH:
          with tc.For_i(0, NMCH, 1, staggered_reset=SR) as mc:
            nc.sync.dma_start(out=k_stage[:],
                              in_=keys_sb[:, ds(mc * NCHUNK, NCHUNK)])
            nc.sync.dma_start(out=xl_a[:],
                              in_=x4[:, ds(mc, 1), :, 0, :])
            nc.sync.dma_start(out=xl_b[:],
                              in_=x4[:, ds(mc, 1), :, 1, :])
            for u in range(4):
                xl_half = (xl_a, xl_b)[u // 2]
                uo = (u % 2) * P
                vps = ps23.tile([P, L], F32, tag="ps23", name=f"vps{u}")
                for j in range(NMCH):
                    nc.tensor.matmul(
                        out=vps[:, j * NCHUNK:(j + 1) * NCHUNK],
                        lhsT=k_stage[:, u * P:(u + 1) * P],
                        rhs=quer_sb[:, j * NCHUNK:(j + 1) * NCHUNK],
                        start=True, stop=True)
                nc.scalar.activation(
                    E_sb[:, u * NMCH * L:(u + 1) * NMCH * L][:, ds(mc * L, L)],
                    vps[:], AF.Exp, scale=2.0 / L,
                    accum_out=s_st[:, u:u + 1])
                nc.vector.reciprocal(out=rs_st[:, u:u + 1],
                                     in_=s_st[:, u:u + 1])
                acc3 = ps23.tile([P, C], F32, tag="ps23", name=f"acc3{u}")
                for dc in range(C // NCHUNK):
                    for c in range(NC_TILES):
                        nc.tensor.matmul(
                            out=acc3[:, dc * NCHUNK:(dc + 1) * NCHUNK],
                            lhsT=xl_half[:, c, uo:uo + P],
                            rhs=wp_sb[:, c * C + dc * NCHUNK:
                                      c * C + (dc + 1) * NCHUNK],
                            start=(c == 0), stop=(c == NC_TILES - 1))
                nc.vector.tensor_scalar_mul(
                    out=yt_sb[:, u * NMCH * C:(u + 1) * NMCH * C]
                        [:, ds(mc * C, C)],
                    in0=acc3[:], scalar1=rs_st[:, u:u + 1])
        ps23.release()

        # ============ L3: out = yTs^T @ E + bp (4 iters) ============
        outp = tc.alloc_tile_pool(name="outp", bufs=2)
        ps4 = tc.alloc_tile_pool(name="ps4", bufs=1, space="PSUM")
        out_v = out_d.rearrange("(n p) l -> p n l", p=P)
        e_v = E_sb.rearrange("p (t m) -> p t m", t=NL_TILES)
        if "3" in PH:
          with tc.For_i(0, NMCH, 1, staggered_reset=SR) as mc:
            for q in range(4):
                nc.sync.dma_start(
                    out=a_stage[:, q * 4:(q + 1) * 4, :],
                    in_=e_v[:, q * 4:(q + 1) * 4, ds(mc * NCHUNK, NCHUNK)])
            accs = [ps4.tile([P, NCHUNK], F32, tag=f"ps4_{d}",
                             name=f"acc4_{d}")
                    for d in range(ND_TILES)]
            for t in range(NL_TILES):
                for d in range(ND_TILES):
                    nc.tensor.matmul(
                        out=accs[d][:],
                        lhsT=yt_sb[:, t * C + d * P:t * C + (d + 1) * P],
                        rhs=a_stage[:, t, :],
                        start=(t == 0), stop=(t == NL_TILES - 1))
            for d in range(ND_TILES):
                o_sb = outp.tile([P, NCHUNK], F32, tag="o")
                nc.vector.tensor_scalar_add(out=o_sb[:], in0=accs[d][:],
                                            scalar1=b_sb[:, 2 + d:3 + d])
                nc.sync.dma_start(out=out_v[:, d, ds(mc * NCHUNK, NCHUNK)],
                                  in_=o_sb[:])
        ps4.release()
        outp.release()

        if rep_ctx is not None:
            rep_ctx.__exit__(None, None, None)

    nc.compile()
    return nc


def _get_nc(rep: int = 1):
    if rep not in _NC_CACHE:
        _NC_CACHE[rep] = build_nc(rep)
    return _NC_CACHE[rep]


def make_in_maps(x, Wk, bk, Wq, bq, Wp, bp):
    bf = ml_dtypes.bfloat16
    x = np.asarray(x, dtype=np.float32)
    # wpT c-tile-major: [128, c*1024 + d]
    wpT = np.ascontiguousarray(np.asarray(Wp, np.float32).T)       # [C, C]
    wp_blob = (wpT.reshape(NC_TILES, P, C).transpose(1, 0, 2)
               .reshape(P, NC_TILES * C).astype(bf))
    wkT = np.asarray(Wk, np.float32).T                             # [C, A]
    wqT = np.asarray(Wq, np.float32).T
    kq_blob = np.concatenate([
        wkT.reshape(NC_TILES, P, A).transpose(1, 0, 2).reshape(P, -1),
        wqT.reshape(NC_TILES, P, A).transpose(1, 0, 2).reshape(P, -1),
    ], axis=1).astype(bf)
    b_blob = np.concatenate([
        np.asarray(bk, np.float32).reshape(P, 1),
        np.asarray(bq, np.float32).reshape(P, 1),
        np.ascontiguousarray(np.asarray(bp, np.float32).reshape(ND_TILES, P).T),
    ], axis=1).astype(np.float32)
    in_maps = []
    for b in range(B):
        # x m-chunk-major: [128, mc*4096 + c*512 + j]
        x_blob = (x[b].reshape(NC_TILES, P, NMCH, NCHUNK)
                  .transpose(1, 2, 0, 3).reshape(P, NMCH * XCH).astype(bf))
        in_maps.append({"x": np.ascontiguousarray(x_blob), "wp": wp_blob,
                        "kq": kq_blob, "b": b_blob})
    return in_maps


def kernel(x, Wk, bk, Wq, bq, Wp, bp):
    nc = _get_nc(1)
    in_maps = make_in_maps(x, Wk, bk, Wq, bq, Wp, bp)
    res = run_bass_kernel_spmd(nc, in_maps, list(range(B)))
    return np.stack([res.results[b]["out"] for b in range(B)]).astype(np.float32)


# revision 3
# speedup vs baseline: 1.0006x; 1.0006x over previous
"""Trainium2 Bass kernel for nn_BasicAttention (B=8, C=1024, L=2048, A=128).

Sharding: data-parallel over batch B — one example per NeuronCore, no
collectives.

Math (per example), associativity avoids any on-device transpose:
    keys    = Wk @ x + bk                      [A, L]
    queries = Wq @ x + bq                      [A, L]
    V       = keys^T @ queries                 [L, L]
    E       = exp(V / (L/2))   (raw exp; logits are ~1e-2 so no max-sub)
    S[l]    = sum_m E[l, m]
    yT      = x^T @ Wp^T       (= (Wp @ x)^T)  [L, C]
    out     = (yT / S)^T @ E + bp              [C, L]

v4 = the original static-PE-address discipline (every matmul operand AP
is compile-time static; register offsets appear only on DMA and
activation APs, which tolerate them) + two structural wins:
  * all GEMMs in bf16 (same PE rate as f32r here, half the bytes), so
  * E = exp(V) lives entirely in SBUF (64 KiB/partition) — phase 2 no
    longer round-trips 32 MB/core through DRAM, and phase 3 stages E
    chunks via cheap SBUF->SBUF DMA instead of DRAM reads.
Phase 2 processes 4 l-tiles per loop body so the x l-block staging DMA
is one contiguous copy amortized over 4x the matmul work, split in two
halves so the next body's staging overlaps this body's tail matmuls.

Layouts (per partition p):
    x_sb  col = mc*4096 + c*512 + j       (mc m-chunk, c c-tile, j in chunk)
    wp_sb col = c*1024 + d_out            (wpT, c-tile-major)
    kq_sb col = c*128 + a (wkT) then +1024 same for wqT
    E_sb  tile l=4*mc+u in slot t=(u*4+mc), col t*2048
    yt_sb tile l=4*mc+u in slot t=(u*4+mc), col t*1024
"""

import os
import sys

for _p in ("/opt/trn_rl_repo", "/root/.axon_site/_ro/trn_rl_repo"):
    if os.path.isdir(_p) and _p not in sys.path:
        sys.path.insert(0, _p)

import numpy as np
import ml_dtypes
from contextlib import ExitStack

from concourse import bass, bacc, mybir, tile
from concourse.bass_utils import run_bass_kernel_spmd

P = 128
B, C, L, A = 8, 1024, 2048, 128
NC_TILES = C // P          # 8 c-tiles
NL_TILES = L // P          # 16 l-tiles
ND_TILES = C // P          # 8 d-tiles
NCHUNK = 512
NMCH = L // NCHUNK         # 4 m-chunks
XCH = NC_TILES * NCHUNK    # x cols per m-chunk (4096)

F32 = mybir.dt.float32
BF16 = mybir.dt.bfloat16
AF = mybir.ActivationFunctionType
ds = bass.ds

_NC_CACHE = {}


def build_nc(rep: int = 1):
    SR = os.environ.get('KERNEL_SR', '1') == '1'
    PH = os.environ.get('BENCH_PHASES', '123')
    REP_SR = os.environ.get('REP_SR', '0') == '1'
    nc = bacc.Bacc(None, target_bir_lowering=False)

    x_d = nc.declare_dram_parameter("x", [P, NMCH * XCH], BF16, isOutput=False)
    wp_d = nc.declare_dram_parameter("wp", [P, NC_TILES * C], BF16, isOutput=False)
    kq_d = nc.declare_dram_parameter("kq", [P, 2 * NC_TILES * A], BF16, isOutput=False)
    b_d = nc.declare_dram_parameter("b", [P, 2 + ND_TILES], F32, isOutput=False)
    out_d = nc.declare_dram_parameter("out", [C, L], F32, isOutput=True)

    with tile.TileContext(nc) as tc, ExitStack() as octx:
        big = octx.enter_context(tc.tile_pool(name="big", bufs=1))
        x_sb = big.tile([P, NMCH * XCH], BF16)
        wp_sb = big.tile([P, NC_TILES * C], BF16)
        kq_sb = big.tile([P, 2 * NC_TILES * A], BF16)
        b_sb = big.tile([P, 2 + ND_TILES], F32)
        keys_sb = big.tile([P, L], BF16)
        quer_sb = big.tile([P, L], BF16)
        E_sb = big.tile([P, NL_TILES * L], BF16)
        yt_sb = big.tile([P, NL_TILES * C], BF16)
        k_stage = big.tile([P, NCHUNK], BF16)
        xl_a = big.tile([P, NC_TILES, 2 * P], BF16)   # u=0,1 halves
        xl_b = big.tile([P, NC_TILES, 2 * P], BF16)   # u=2,3 halves
        a_stage = big.tile([P, NL_TILES, NCHUNK], BF16)
        s_st = big.tile([P, NMCH], F32)
        rs_st = big.tile([P, NMCH], F32)

        nc.sync.dma_start(out=kq_sb[:], in_=kq_d[:])
        nc.sync.dma_start(out=b_sb[:], in_=b_d[:])
        for mc in range(NMCH):
            nc.sync.dma_start(out=x_sb[:, mc * XCH:(mc + 1) * XCH],
                              in_=x_d[:, mc * XCH:(mc + 1) * XCH])
        nc.sync.dma_start(out=wp_sb[:], in_=wp_d[:])

        # x viewed as [p, mc, c, u-half, 256]
        x4 = x_sb.rearrange("p (m c h q) -> p m c h q",
                            m=NMCH, c=NC_TILES, h=2)

        rep_ctx = (tc.For_i(0, rep, 1, staggered_reset=REP_SR)
                   if rep > 1 else None)
        if rep_ctx is not None:
            rep_ctx.__enter__()

        # ============ L1: K/Q projections (4 iters) ============
        ps1 = tc.alloc_tile_pool(name="ps1", bufs=2, space="PSUM")
        if "1" in PH:
          with tc.For_i(0, NMCH, 1, staggered_reset=SR) as mc:
            for woff, bcol, o_sb in ((0, 0, keys_sb), (C, 1, quer_sb)):
                acc = ps1.tile([P, NCHUNK], F32, tag="ps1",
                               name="accK" if woff == 0 else "accQ")
                for c in range(NC_TILES):
                    nc.tensor.matmul(
                        out=acc[:],
                        lhsT=kq_sb[:, woff + c * A:woff + (c + 1) * A],
                        rhs=x_sb[:, ds(mc * XCH + c * NCHUNK, NCHUNK)],
                        start=(c == 0), stop=(c == NC_TILES - 1))
                nc.scalar.activation(
                    o_sb[:, ds(mc * NCHUNK, NCHUNK)], acc[:],
                    AF.Identity, bias=b_sb[:, bcol:bcol + 1])
        ps1.release()

        # ==== L2: values + exp + yT (4 iters, 4 l-tiles each) ====
        ps23 = tc.alloc_tile_pool(name="ps23", bufs=2, space="PSUM")
        if "2" in PH:
          with tc.For_i(0, NMCH, 1, staggered_reset=SR) as mc:
            nc.sync.dma_start(out=k_stage[:],
                              in_=keys_sb[:, ds(mc * NCHUNK, NCHUNK)])
            nc.scalar.dma_start(out=xl_a[:],
                                in_=x4[:, ds(mc, 1), :, 0, :])
            nc.scalar.dma_start(out=xl_b[:],
                                in_=x4[:, ds(mc, 1), :, 1, :])
            for u in range(4):
                xl_half = (xl_a, xl_b)[u // 2]
                uo = (u % 2) * P
                vps = ps23.tile([P, L], F32, tag="ps23", name=f"vps{u}")
                for j in range(NMCH):
                    nc.tensor.matmul(
                        out=vps[:, j * NCHUNK:(j + 1) * NCHUNK],
                        lhsT=k_stage[:, u * P:(u + 1) * P],
                        rhs=quer_sb[:, j * NCHUNK:(j + 1) * NCHUNK],
                        start=True, stop=True)
                nc.scalar.activation(
                    E_sb[:, u * NMCH * L:(u + 1) * NMCH * L][:, ds(mc * L, L)],
                    vps[:], AF.Exp, scale=2.0 / L,
                    accum_out=s_st[:, u:u + 1])
                nc.vector.reciprocal(out=rs_st[:, u:u + 1],
                                     in_=s_st[:, u:u + 1])
                acc3 = ps23.tile([P, C], F32, tag="ps23", name=f"acc3{u}")
                for dc in range(C // NCHUNK):
                    for c in range(NC_TILES):
                        nc.tensor.matmul(
                            out=acc3[:, dc * NCHUNK:(dc + 1) * NCHUNK],
                            lhsT=xl_half[:, c, uo:uo + P],
                            rhs=wp_sb[:, c * C + dc * NCHUNK:
                                      c * C + (dc + 1) * NCHUNK],
                            start=(c == 0), stop=(c == NC_TILES - 1))
                nc.vector.tensor_scalar_mul(
                    out=yt_sb[:, u * NMCH * C:(u + 1) * NMCH * C]
                        [:, ds(mc * C, C)],
                    in0=acc3[:], scalar1=rs_st[:, u:u + 1])
        ps23.release()

        # ============ L3: out = yTs^T @ E + bp (4 iters) ============
        outp = tc.alloc_tile_pool(name="outp", bufs=2)
        ps4 = tc.alloc_tile_pool(name="ps4", bufs=1, space="PSUM")
        out_v = out_d.rearrange("(n p) l -> p n l", p=P)
        e_v = E_sb.rearrange("p (t m) -> p t m", t=NL_TILES)
        if "3" in PH:
          with tc.For_i(0, NMCH, 1, staggered_reset=SR) as mc:
            for q in range(4):
                nc.sync.dma_start(
                    out=a_stage[:, q * 4:(q + 1) * 4, :],
                    in_=e_v[:, q * 4:(q + 1) * 4, ds(mc * NCHUNK, NCHUNK)])
            accs = [ps4.tile([P, NCHUNK], F32, tag=f"ps4_{d}",
                             name=f"acc4_{d}")
                    for d in range(ND_TILES)]
            for t in range(NL_TILES):
                for d in range(ND_TILES):
                    nc.tensor.matmul(
                        out=accs[d][:],
                        lhsT=yt_sb[:, t * C + d * P:t * C + (d + 1) * P],
                        rhs=a_stage[:, t, :],
                        start=(t == 0), stop=(t == NL_TILES - 1))
            for d in range(ND_TILES):
                o_sb = outp.tile([P, NCHUNK], F32, tag="o")
                nc.vector.tensor_scalar_add(out=o_sb[:], in0=accs[d][:],
                                            scalar1=b_sb[:, 2 + d:3 + d])
                nc.sync.dma_start(out=out_v[:, d, ds(mc * NCHUNK, NCHUNK)],
                                  in_=o_sb[:])
        ps4.release()
        outp.release()

        if rep_ctx is not None:
            rep_ctx.__exit__(None, None, None)

    nc.compile()
    return nc


def _get_nc(rep: int = 1):
    if rep not in _NC_CACHE:
        _NC_CACHE[rep] = build_nc(rep)
    return _NC_CACHE[rep]


def make_in_maps(x, Wk, bk, Wq, bq, Wp, bp):
    bf = ml_dtypes.bfloat16
    x = np.asarray(x, dtype=np.float32)
    # wpT c-tile-major: [128, c*1024 + d]
    wpT = np.ascontiguousarray(np.asarray(Wp, np.float32).T)       # [C, C]
    wp_blob = (wpT.reshape(NC_TILES, P, C).transpose(1, 0, 2)
               .reshape(P, NC_TILES * C).astype(bf))
    wkT = np.asarray(Wk, np.float32).T                             # [C, A]
    wqT = np.asarray(Wq, np.float32).T
    kq_blob = np.concatenate([
        wkT.reshape(NC_TILES, P, A).transpose(1, 0, 2).reshape(P, -1),
        wqT.reshape(NC_TILES, P, A).transpose(1, 0, 2).reshape(P, -1),
    ], axis=1).astype(bf)
    b_blob = np.concatenate([
        np.asarray(bk, np.float32).reshape(P, 1),
        np.asarray(bq, np.float32).reshape(P, 1),
        np.ascontiguousarray(np.asarray(bp, np.float32).reshape(ND_TILES, P).T),
    ], axis=1).astype(np.float32)
    in_maps = []
    for b in range(B):
        # x m-chunk-major: [128, mc*4096 + c*512 + j]
        x_blob = (x[b].reshape(NC_TILES, P, NMCH, NCHUNK)
                  .transpose(1, 2, 0, 3).reshape(P, NMCH * XCH).astype(bf))
        in_maps.append({"x": np.ascontiguousarray(x_blob), "wp": wp_blob,
                        "kq": kq_blob, "b": b_blob})
    return in_maps


def kernel(x, Wk, bk, Wq, bq, Wp, bp):
    nc = _get_nc(1)
    in_maps = make_in_maps(x, Wk, bk, Wq, bq, Wp, bp)
    res = run_bass_kernel_spmd(nc, in_maps, list(range(B)))
    return np.stack([res.results[b]["out"] for b in range(B)]).astype(np.float32)
